# revision 5
# baseline (speedup 1.0000x reference)
"""BuildCostVolume kernel for 8 Trainium2 NeuronCores.

Decomposition: the 9 strided dilated convs (disparities d=-4..4) reduce to
729 taps (d,u,v): cost[b,co,d,h,w] = sum_{ci,u,v} Wd[d][co,ci,u,v] *
X[b,ci,u,v, h+d(4-u), w+d(4-v)] where X is the view-decomposed light field
(X[b,ci,u,v,h,w] = x[b,ci,9h+u,9w+v], zero outside) and Wd flips (u,v) for
d>0.  Each tap is a K=64(ci) x M=64(co) matmul over spatial positions.

Sharding: core = (batch b, h-half).  Each core holds all 81 views' h-windows
(zero-padded to uniform per-u heights) resident in SBUF as 41 view-pair
tiles ([128, R*48] bf16: two views stacked on partition halves).  Taps run
as 4-way concurrent matmuls via tile_position (2 row-groups x 2 col-groups),
accumulating per (d, 8-row subchunk) into a PSUM bank laid out w-major so
w-clipped boxes stay contiguous 2D APs.  DVE adds the two col-half partial
sums; results DMA out w-major and the host transposes back.
"""

import numpy as np
import ml_dtypes

A = 9           # angular resolution
H = 48          # spatial h/w per view
C = 64          # channels (ci = co = 64)
B = 4           # batch
ND = 9          # disparities -4..4
HH = 24         # h rows per core (half)
SUB = 8         # output h rows per psum accumulation group
NSUB = HH // SUB
NSLOT = 41      # weight slots per row-half per d
N_CORES = 8

BF16 = ml_dtypes.bfloat16


def _geometry():
    """Static tap/tile geometry shared by host packing and device program."""
    pairs = []            # (viewA, viewB-or-None, R)
    for v in range(A):
        for u in range(4):
            pairs.append(((u, v), (8 - u, v), HH + 8 * (4 - u)))
    for k in range(4):
        pairs.append(((4, k), (4, k + 5), HH))
    pairs.append(((4, 4), None, HH))

    view_loc = {}
    offs = []
    off = 0
    for j, (va, vb, R) in enumerate(pairs):
        view_loc[va] = (j, 0)
        if vb is not None:
            view_loc[vb] = (j, 1)
        offs.append(off)
        off += R * H
    F = off

    # per-d emission list: taps cycled over the 4 tile positions, each
    # position seeded with a full-width (v=4) tap so the first matmul per
    # col-half clears its whole psum box.
    taps_by_d = []
    for d in range(-4, 5):
        seeds = {(0, 0): (0, 4), (0, 1): (1, 4), (1, 0): (8, 4), (1, 1): (7, 4)}
        used = set(seeds.values())
        lists = {p: [s] for p, s in seeds.items()}
        nch = {0: 1, 1: 1}
        for u in range(A):
            for v in range(A):
                if (u, v) in used:
                    continue
                rh = view_loc[(u, v)][1]
                ch = 0 if nch[rh] <= nch[1 - rh] else 1
                # balance col-halves within each row-half
                c0 = len(lists[(rh, 0)])
                c1 = len(lists[(rh, 1)])
                ch = 0 if c0 <= c1 else 1
                lists[(rh, ch)].append((u, v))
        order = []
        idx = {p: 0 for p in lists}
        keys = [(0, 0), (0, 1), (1, 0), (1, 1)]
        while any(idx[p] < len(lists[p]) for p in keys):
            for p in keys:
                if idx[p] < len(lists[p]):
                    u, v = lists[p][idx[p]]
                    idx[p] += 1
                    order.append((u, v, p[0], p[1]))
        # weight slot per row-half, in emission order
        slot_ctr = {0: 0, 1: 0}
        taps = []
        for (u, v, rh, ch) in order:
            s = slot_ctr[rh]
            slot_ctr[rh] += 1
            taps.append((u, v, rh, ch, s))
        taps_by_d.append(taps)

    return pairs, view_loc, offs, F, taps_by_d


_PAIRS, _VIEW_LOC, _OFFS, _F, _TAPS = _geometry()
_NC_CACHE = {}


def _build_nc():
    import concourse.bacc as bacc
    import concourse.mybir as mybir
    import concourse.tile as tile

    nc = bacc.Bacc(None, target_bir_lowering=False)
    xwin_d = nc.dram_tensor("xwin", [128, _F], mybir.dt.bfloat16,
                            kind="ExternalInput")
    wp_d = nc.dram_tensor("wpack", [128, ND * NSLOT * C], mybir.dt.bfloat16,
                          kind="ExternalInput")
    out_d = nc.dram_tensor("out", [C, ND * NSUB * SUB * H], mybir.dt.float32,
                           kind="ExternalOutput")

    with tile.TileContext(nc) as tc:
        with tc.tile_pool(name="xw", bufs=1) as xpool, \
             tc.tile_pool(name="wp", bufs=2) as wpool, \
             tc.tile_pool(name="ps", bufs=3, space="PSUM") as ppool, \
             tc.tile_pool(name="ob", bufs=4) as opool:

            xtiles = []
            xviews = []
            for j, (va, vb, R) in enumerate(_PAIRS):
                t = xpool.tile([128, R * H], mybir.dt.bfloat16, tag=f"x{j}")
                nc.sync.dma_start(out=t[:], in_=xwin_d[:, _OFFS[j]:_OFFS[j] + R * H])
                xtiles.append(t)
                # [p, w, r]: w outer / h inner to match w-major psum layout
                xviews.append(t[:].rearrange("p (r w) -> p w r", r=R, w=H))

            for di in range(ND):
                d = di - 4
                wt = wpool.tile([128, NSLOT * C], mybir.dt.bfloat16, tag="wt")
                nc.sync.dma_start(
                    out=wt[:],
                    in_=wp_d[:, di * NSLOT * C:(di + 1) * NSLOT * C])
                taps = _TAPS[di]
                last_pos = {}
                for i, (u, v, rh, ch, s) in enumerate(taps):
                    last_pos[(rh, ch)] = i
                for sub in range(NSUB):
                    # concurrent tile-position matmuls must write DISJOINT
                    # psum regions (same-region row-tile accumulation crashes
                    # the exec unit): rh0 -> ptA, rh1 -> ptB, ch picks the
                    # partition half within each bank.
                    ptA = ppool.tile([128, SUB * H], mybir.dt.float32, tag="ptA")
                    ptB = ppool.tile([128, SUB * H], mybir.dt.float32, tag="ptB")
                    pts = (ptA, ptB)
                    started = set()
                    for i, (u, v, rh, ch, s) in enumerate(taps):
                        j, half = _VIEW_LOC[(u, v)]
                        assert half == rh
                        au = abs(4 - u)
                        row0 = sub * SUB + d * (4 - u) + 4 * au
                        sv = d * (4 - v)
                        wlo = max(0, -sv)
                        whi = min(H, H - sv)
                        rhs = xviews[j][rh * 64:(rh + 1) * 64,
                                        wlo + sv:whi + sv,
                                        row0:row0 + SUB]
                        lhsT = wt[rh * 64:(rh + 1) * 64, s * C:(s + 1) * C]
                        outap = pts[rh][ch * 64:(ch + 1) * 64,
                                        wlo * SUB:whi * SUB]
                        nc.tensor.matmul(
                            outap, lhsT, rhs,
                            start=((rh, ch) not in started),
                            stop=(i == last_pos[(rh, ch)]),
                            tile_position=(rh * 64, ch * 64),
                            skip_group_check=True,
                        )
                        started.add((rh, ch))
                    ot = opool.tile([64, SUB * H], mybir.dt.float32, tag="ot")
                    # walrus: only one non-scalar input may read PSUM per op
                    nc.vector.tensor_copy(ot[:], ptA[0:64, :])
                    nc.vector.tensor_add(ot[:], ot[:], ptA[64:128, :])
                    nc.vector.tensor_add(ot[:], ot[:], ptB[0:64, :])
                    nc.vector.tensor_add(ot[:], ot[:], ptB[64:128, :])
                    seg = (di * NSUB + sub) * SUB * H
                    nc.sync.dma_start(out=out_d[:, seg:seg + SUB * H],
                                      in_=ot[:])

    nc.finalize()
    return nc


def get_nc():
    if "nc" not in _NC_CACHE:
        _NC_CACHE["nc"] = _build_nc()
    return _NC_CACHE["nc"]


def prepare_inputs(x, W):
    """Host-side packing: per-core xwin [128,F] bf16 + shared wpack."""
    x = np.asarray(x, dtype=np.float32)
    W = np.asarray(W, dtype=np.float32)
    # X5[b,u,v,ci,h,w]
    X5 = np.ascontiguousarray(
        x.reshape(B, C, H, A, H, A).transpose(0, 3, 5, 1, 2, 4)
    ).astype(BF16)

    xwins = []
    for core in range(N_CORES):
        b, hh = divmod(core, 2)
        h0 = hh * HH
        xw = np.zeros((128, _F), dtype=BF16)
        for j, (va, vb, R) in enumerate(_PAIRS):
            for half, view in ((0, va), (1, vb)):
                if view is None:
                    continue
                u, v = view
                lo = h0 - 4 * abs(4 - u)
                vs = max(0, lo)
                ve = min(H, lo + R)
                blk = X5[b, u, v, :, vs:ve, :]  # [64, ve-vs, 48]
                dst = xw[half * 64:(half + 1) * 64,
                         _OFFS[j]:_OFFS[j] + R * H].reshape(64, R, H)
                dst[:, vs - lo:ve - lo, :] = blk
        xwins.append(xw)

    wpack = np.zeros((128, ND * NSLOT * C), dtype=BF16)
    Wb = W.astype(BF16)
    for di in range(ND):
        d = di - 4
        for (u, v, rh, ch, s) in _TAPS[di]:
            kh, kw = (u, v) if d <= 0 else (8 - u, 8 - v)
            # lhsT[ci, co] = Wd[co, ci, kh, kw]
            wpack[rh * 64:(rh + 1) * 64,
                  (di * NSLOT + s) * C:(di * NSLOT + s + 1) * C] = \
                Wb[:, :, kh, kw].T
    return xwins, wpack


def assemble_output(results):
    """results: list of 8 dicts with 'out' [64, ND*NSUB*SUB*H] fp32."""
    full = np.empty((B, C, ND, H, H), dtype=np.float32)
    for core in range(N_CORES):
        b, hh = divmod(core, 2)
        oc = np.asarray(results[core]["out"]).reshape(C, ND, NSUB, H, SUB)
        # [co, d, sub, w, h] -> [co, d, sub, h, w]
        oc = oc.transpose(0, 1, 2, 4, 3).reshape(C, ND, HH, H)
        full[b, :, :, hh * HH:(hh + 1) * HH, :] = oc
    return full


def kernel(x, W):
    from concourse.bass_utils import run_bass_kernel_spmd

    nc = get_nc()
    xwins, wpack = prepare_inputs(x, W)
    in_maps = [{"xwin": xwins[c], "wpack": wpack} for c in range(N_CORES)]
    res = run_bass_kernel_spmd(nc, in_maps, core_ids=list(range(N_CORES)))
    return assemble_output(res.results)
